# revision 32
# baseline (speedup 1.0000x reference)
"""Trainium2 Bass kernel for the YOLO-style DetectionLoss.

Full inputs in, full (scalar) output out. Internally:
  - Only the conf channels (a*8+4, i.e. 3 of 24 channels) need a full-tensor
    pass: loss_conf = mean((sigmoid(conf) - m)^2). Decompose as
       sum_all sigmoid(conf)^2  +  sum_masked [(sig-1)^2 - sig^2]
    so the bulk device work is an 8-way batch-sharded sigmoid-square-reduce
    over pred[:, 4::8] (1.23 MB/core instead of 9.8 MB/core for full pred).
  - The masked box/cls/conf-correction terms only touch the <=512 target
    cells; those 24 values/cell are gathered host-side (pure indexing) and
    evaluated on-device in one small (24, 3*ncells) block per core laid out
    as [u | q | T] along the free axis. To keep the ACT engine on a single
    function table (table switches cost ~1.3us), exp(v) is computed as
    1/sigmoid(-v) - 1 via the DVE reciprocal:
       u = v on sigmoid-cols else -100    -> sig(u) = sig(v) or 0
       q = -v on exp-cols   else +100     -> 1/sig(q) - 1 = e^v or 0
       F = (1/sig(q) - 1) + sig(u);  per-channel-row reductions:
       r1[row] = sum_cells (F - T)^2 ;  r2[row] = sum_cells F^2
  - Host combines the 8 cores' partial sums and applies the final divisions.

Perf notes (measured on trn2 via ntff profiles):
  - The stock TileContext tail (drain + EVSEM butterfly + sem clear +
    butterfly) serializes every engine at the end, and the runtime epilogue
    then resets the full 256-sem space (~50 sems/engine, ~60-115ns each)
    AFTER that rendezvous. Skipping the Tile tail entirely lets idle
    engines run their epilogue resets during the kernel body. The runtime
    epilogue re-zeroes every semaphore, so re-execution stays correct.
  - A second act-table load (set 0) is inserted at entry by the fixpoint
    pass even though only the sigmoid table (set 2) is used; it is dropped.
"""

import numpy as np

A = 3
NUM_CLS = 3
B, C, H, W = 32, 24, 160, 160
HW = H * W
M = 8            # cores
BPC = B // M     # batches per core
P = 128
CONF_ELEMS = BPC * A * HW        # 307200 per core
FREE = CONF_ELEMS // P           # 2400
NEG = -100.0                     # sigmoid(-100) == 0, sigmoid(+100) == 1 in f32

CHUNKS = (320, 832, 832, 416)   # small first chunk hides the first DMA receipt;
                                # small last chunk shortens the DVE tail
TAIL_MODE = 2      # 0 = stock Tile tail; 1 = sem-only barrier; 2 = no tail
DROP_TABLE0 = True

TRACE = False        # test harness can flip this to get a profile
LAST = None          # BassKernelResults of the most recent run

_PROGRAM_CACHE = {}


def _make_tile_context(nc):
    import concourse.tile as tile
    from concourse.vector_clock import ScopedClock

    class _FastTailTileContext(tile.TileContext):
        def _drain_and_barrier(self, tick_clock, wait_clock):
            if TAIL_MODE == 0:
                return super()._drain_and_barrier(tick_clock, wait_clock)
            if TAIL_MODE == 1:
                drain_inst = self.nc.sync.drain()
                wait_clock.add_sem_waits(
                    drain_inst.ins, ScopedClock({None: tick_clock.global_clock})
                )
                self.nc.all_engine_barrier(sem_only=True)
                popped = self.nc._tile_sem_poison_stack.pop()
                assert popped is self._sem_poison
                self.nc.clear_and_free_semaphores(
                    list(self.sems.allocated().values())
                )
                return
            # TAIL_MODE == 2: no in-kernel tail at all. In-body semaphores
            # already order every data dependency (incl. the output DMA);
            # NEFF completion itself waits for engine streams + DMA queues,
            # and the runtime epilogue zeroes the whole semaphore space.
            popped = self.nc._tile_sem_poison_stack.pop()
            assert popped is self._sem_poison

    return _FastTailTileContext(nc)


def _make_bacc():
    from concourse import bacc, mybir

    class _Bacc(bacc.Bacc):
        def __init__(self, *a, **kw):
            # Skip the const-memset all-engine barrier Bass.__init__ emits
            # (~1us on the critical path). The only consumer of those const
            # tiles here is the activation bias, which we replace with a
            # tile-tracked zero tile inside the TileContext.
            self._skip_init_barrier = True
            super().__init__(*a, **kw)
            self._skip_init_barrier = False

        def all_engine_barrier(self, *, sem_only: bool = False):
            if getattr(self, "_skip_init_barrier", False):
                return
            super().all_engine_barrier(sem_only=sem_only)

        def insert_act_table_loads(self):
            super().insert_act_table_loads()
            if not DROP_TABLE0:
                return
            # The entry-state fixpoint conservatively loads table set 0, but
            # every activation here is from the sigmoid set, which gets its
            # own load. Drop the set-0 load (1.28us on the ACT engine). Also
            # drop the const-* memsets whose only consumer (activation bias)
            # was replaced by an in-context zero tile.
            for blk in self.main_func.blocks:
                keep = []
                for inst in blk.instructions:
                    if (
                        isinstance(inst, mybir.InstLoadActFuncSet)
                        and inst.act_func_set_id == 0
                        and not (
                            inst.sync_info
                            and (inst.sync_info.on_wait or inst.sync_info.on_update)
                        )
                    ):
                        continue
                    if (
                        isinstance(inst, mybir.InstMemset)
                        and inst.outs
                        and str(inst.outs[0].memref).startswith("const-")
                        and not (
                            inst.sync_info
                            and (inst.sync_info.on_wait or inst.sync_info.on_update)
                        )
                    ):
                        continue
                    keep.append(inst)
                blk.instructions[:] = keep

    return _Bacc("TRN2", target_bir_lowering=False, debug=False, num_devices=M)


def _build_program(ncells_pad, chunks):
    from concourse import mybir

    f32 = mybir.dt.float32
    bf16 = mybir.dt.bfloat16
    Act = mybir.ActivationFunctionType
    Alu = mybir.AluOpType

    nc = _make_bacc()
    nchunks = len(chunks)

    NC = ncells_pad
    conf_t = nc.dram_tensor("conf", [P, FREE], bf16, kind="ExternalInput")
    # columns [0:NC]=u, [NC:2NC]=q, [2NC:3NC]=T; rows 24-127 are padding so
    # the DMA uses the fast full-128-partition descriptor path
    tin_t = nc.dram_tensor("tin", [P, 3 * NC], f32, kind="ExternalInput")
    oall_t = nc.dram_tensor("oall", [P, nchunks + 2], f32, kind="ExternalOutput")

    assert sum(chunks) == FREE

    with _make_tile_context(nc) as tc:
        with (
            tc.tile_pool(name="x", bufs=4) as xp,
            tc.tile_pool(name="s", bufs=3) as sp,
            tc.tile_pool(name="scr", bufs=2) as scrp,
            tc.tile_pool(name="acc", bufs=1) as accp,
            tc.tile_pool(name="tgt", bufs=1) as tp,
        ):
            acc = accp.tile([P, nchunks + 2], f32)

            # zero bias tile for every activation (replaces the global const
            # tile whose init barrier we skip; Tile orders the memset first)
            zb = accp.tile([P, 1], f32)
            nc.gpsimd.memset(zb[:], 0.0)

            # ---- masked cells: one (128, 3*NC) block, columns [u | q | T] ----
            t24 = tp.tile([P, 3 * NC], f32)
            nc.scalar.dma_start(t24[:], tin_t.ap()[:])
            # dummy first activation: binds the auto-inserted ACT table load
            # to the (early) zb memset instead of the part2 DMA receipt
            dum = tp.tile([P, 1], f32)
            nc.scalar.activation(dum[:], zb[:], Act.Sigmoid, bias=zb[:])
            sg = tp.tile([P, 2 * NC], f32)
            nc.scalar.activation(sg[:], t24[:, 0:2 * NC], Act.Sigmoid, bias=zb[:])
            rc = tp.tile([24, NC], f32)
            nc.vector.reciprocal_approx_fast(rc[:], sg[0:24, NC:2 * NC])
            fm = tp.tile([24, NC], f32)
            nc.vector.scalar_tensor_tensor(
                out=fm[:], in0=rc[:], scalar=-1.0, in1=sg[0:24, 0:NC],
                op0=Alu.add, op1=Alu.add)
            dm = tp.tile([24, NC], f32)
            nc.vector.scalar_tensor_tensor(
                out=dm[:], in0=fm[:], scalar=0.0, in1=t24[0:24, 2 * NC:3 * NC],
                op0=Alu.add, op1=Alu.subtract)
            t1 = tp.tile([24, NC], f32)
            nc.vector.scalar_tensor_tensor(
                out=t1[:], in0=dm[:], scalar=0.0, in1=dm[:],
                op0=Alu.add, op1=Alu.mult,
                accum_out=acc[0:24, nchunks:nchunks + 1])
            t2 = tp.tile([24, NC], f32)
            nc.vector.scalar_tensor_tensor(
                out=t2[:], in0=fm[:], scalar=0.0, in1=fm[:],
                op0=Alu.add, op1=Alu.mult,
                accum_out=acc[0:24, nchunks + 1:nchunks + 2])

            # ---- bulk: sum of sigmoid(conf)^2, chunked over the free dim ----
            # chunk 0's square runs on ACT (Square shares the sigmoid table)
            # to balance ACT vs DVE engine time; the rest square on DVE
            col = 0
            for i, cw in enumerate(chunks):
                x = xp.tile([P, cw], bf16, tag="x")
                nc.sync.dma_start(x[:], conf_t.ap()[:, col:col + cw])
                s = sp.tile([P, cw], bf16, tag="s")
                nc.scalar.activation(s[:], x[:], Act.Sigmoid, bias=zb[:])
                sq = scrp.tile([P, cw], bf16, tag="sq")
                if i == 0:
                    nc.scalar.activation(
                        sq[:], s[:], Act.Square, bias=zb[:],
                        accum_out=acc[:, i:i + 1])
                else:
                    nc.vector.scalar_tensor_tensor(
                        out=sq[:], in0=s[:], scalar=0.0, in1=s[:],
                        op0=Alu.add, op1=Alu.mult, accum_out=acc[:, i:i + 1])
                col += cw

            nc.sync.dma_start(oall_t.ap()[:], acc[:])

    nc.compile()
    return nc


def _get_program(ncells_pad, chunks):
    key = (ncells_pad, chunks)
    if key not in _PROGRAM_CACHE:
        _PROGRAM_CACHE[key] = _build_program(ncells_pad, chunks)
    return _PROGRAM_CACHE[key]


def kernel(pred, targets):
    global LAST
    from concourse.bass_utils import run_bass_kernel_spmd

    pred = np.ascontiguousarray(np.asarray(pred, dtype=np.float32))
    targets = np.asarray(targets, dtype=np.float32)
    assert pred.shape == (B, C, H, W), pred.shape
    N = targets.shape[0]

    # ---- host: parse targets, dedupe cells (last writer wins) ----
    b = targets[:, 0].astype(np.int32)
    c = targets[:, 1].astype(np.int32)
    gix = (targets[:, 2] * W).astype(np.int32)
    giy = (targets[:, 3] * H).astype(np.int32)
    valid = (gix < W) & (giy < H) & (gix >= 0) & (giy >= 0) & (b >= 0) & (b < B)

    cell_map = {}
    for i in range(N):
        if valid[i]:
            cell_map[(int(b[i]), int(giy[i]), int(gix[i]))] = i
    n_cells = len(cell_map)
    n = 3.0 * n_cells

    per_core = [[] for _ in range(M)]
    for (bb, yy, xx), i in cell_map.items():
        per_core[bb // BPC].append((bb, yy, xx, i))

    max_cells = max((len(pc) for pc in per_core), default=0)
    ncells_pad = max(32, ((max_cells + 31) // 32) * 32)

    # ---- host: build per-core shards ----
    pr = pred.reshape(B, A, 8, H, W)
    conf_all = pr[:, :, 4, :, :]  # (B, A, H, W)

    SIG_COL = np.array([k in (0, 1, 4, 5, 6, 7) for k in range(8)] * A)  # (24,)

    import ml_dtypes
    NC = ncells_pad
    in_maps = []
    for m in range(M):
        shard = np.ascontiguousarray(
            conf_all[m * BPC:(m + 1) * BPC]).reshape(P, FREE).astype(
                ml_dtypes.bfloat16)

        cells = per_core[m]
        tin = np.empty((P, 3 * NC), np.float32)
        tin[:, 0:NC] = NEG      # u pad -> sig = 0
        tin[:, NC:2 * NC] = -NEG  # q pad -> sig = 1 -> 1/sig - 1 = 0
        tin[:, 2 * NC:] = 0.0   # T pad
        if cells:
            bbs = np.array([e[0] for e in cells])
            yys = np.array([e[1] for e in cells])
            xxs = np.array([e[2] for e in cells])
            idx = np.array([e[3] for e in cells])
            vals = pred[bbs, :, yys, xxs].T  # (24, ncells)
            ncol = len(cells)
            tin[0:24, 0:ncol] = np.where(SIG_COL[:, None], vals, NEG)
            tin[0:24, NC:NC + ncol] = np.where(SIG_COL[:, None], -NEG, -vals)
            boxes = targets[idx, 2:6].T  # (4, ncells): gx, gy, gw, gh
            onehot = np.zeros((NUM_CLS, ncol), np.float32)
            ci = c[idx]
            ok = (ci >= 0) & (ci < NUM_CLS)
            onehot[ci[ok], np.nonzero(ok)[0]] = 1.0
            t0 = 2 * NC
            for a in range(A):
                tin[a * 8 + 0:a * 8 + 4, t0:t0 + ncol] = boxes
                tin[a * 8 + 4, t0:t0 + ncol] = 1.0
                tin[a * 8 + 5:a * 8 + 8, t0:t0 + ncol] = onehot
        in_maps.append({"conf": shard, "tin": tin})

    # ---- device ----
    nchunks = len(CHUNKS)
    nc = _get_program(ncells_pad, CHUNKS)
    res = run_bass_kernel_spmd(nc, in_maps, list(range(M)), trace=TRACE)
    LAST = res

    # ---- host: combine ----
    S2 = 0.0
    r1_tot = np.zeros(24, np.float64)
    r2_tot = np.zeros(24, np.float64)
    for m in range(M):
        out = res.results[m]["oall"].astype(np.float64)
        S2 += out[:, :nchunks].sum()
        r1_tot += out[0:24, nchunks]
        r2_tot += out[0:24, nchunks + 1]

    box_rows = [a * 8 + k for a in range(A) for k in range(4)]
    conf_rows = [a * 8 + 4 for a in range(A)]
    cls_rows = [a * 8 + k for a in range(A) for k in range(5, 8)]

    box_sum = r1_tot[box_rows].sum()
    cls_sum = r1_tot[cls_rows].sum()
    conf_corr = (r1_tot[conf_rows] - r2_tot[conf_rows]).sum()

    with np.errstate(divide="ignore", invalid="ignore"):
        loss_box = box_sum / (n * 4.0)
        loss_conf = (S2 + conf_corr) / float(B * A * HW)
        loss_cls = cls_sum / (n * NUM_CLS)
        total = 5.0 * loss_box + loss_conf + loss_cls
    return np.asarray(total, dtype=np.float32)


# revision 33
# speedup vs baseline: 1.1188x; 1.1188x over previous
"""Trainium2 Bass kernel for the YOLO-style DetectionLoss.

Full inputs in, full (scalar) output out. Internally:
  - Only the conf channels (a*8+4, i.e. 3 of 24 channels) need a full-tensor
    pass: loss_conf = mean((sigmoid(conf) - m)^2). Decompose as
       sum_all sigmoid(conf)^2  +  sum_masked [(sig-1)^2 - sig^2]
    so the bulk device work is an 8-way batch-sharded sigmoid-square-reduce
    over pred[:, 4::8] (1.23 MB/core instead of 9.8 MB/core for full pred).
  - The masked box/cls/conf-correction terms only touch the <=512 target
    cells; those 24 values/cell are gathered host-side (pure indexing) and
    evaluated on-device in one small (24, 3*ncells) block per core laid out
    as [u | q | T] along the free axis. To keep the ACT engine on a single
    function table (table switches cost ~1.3us), exp(v) is computed as
    1/sigmoid(-v) - 1 via the DVE reciprocal:
       u = v on sigmoid-cols else -100    -> sig(u) = sig(v) or 0
       q = -v on exp-cols   else +100     -> 1/sig(q) - 1 = e^v or 0
       F = (1/sig(q) - 1) + sig(u);  per-channel-row reductions:
       r1[row] = sum_cells (F - T)^2 ;  r2[row] = sum_cells F^2
  - Host combines the 8 cores' partial sums and applies the final divisions.

Perf notes (measured on trn2 via ntff profiles):
  - The stock TileContext tail (drain + EVSEM butterfly + sem clear +
    butterfly) serializes every engine at the end, and the runtime epilogue
    then resets the full 256-sem space (~50 sems/engine, ~60-115ns each)
    AFTER that rendezvous. Skipping the Tile tail entirely lets idle
    engines run their epilogue resets during the kernel body. The runtime
    epilogue re-zeroes every semaphore, so re-execution stays correct.
  - A second act-table load (set 0) is inserted at entry by the fixpoint
    pass even though only the sigmoid table (set 2) is used; it is dropped.
"""

import numpy as np

A = 3
NUM_CLS = 3
B, C, H, W = 32, 24, 160, 160
HW = H * W
M = 8            # cores
BPC = B // M     # batches per core
P = 128
CONF_ELEMS = BPC * A * HW        # 307200 per core
FREE = CONF_ELEMS // P           # 2400
NEG = -100.0                     # sigmoid(-100) == 0, sigmoid(+100) == 1 in f32

CHUNKS = (320, 832, 832, 416)   # small first chunk hides the first DMA receipt;
                                # small last chunk shortens the DVE tail
TAIL_MODE = 2      # 0 = stock Tile tail; 1 = sem-only barrier; 2 = no tail
DROP_TABLE0 = True

TRACE = False        # test harness can flip this to get a profile
LAST = None          # BassKernelResults of the most recent run

_PROGRAM_CACHE = {}


def _make_tile_context(nc):
    import concourse.tile as tile
    from concourse.vector_clock import ScopedClock

    class _FastTailTileContext(tile.TileContext):
        def _drain_and_barrier(self, tick_clock, wait_clock):
            if TAIL_MODE == 0:
                return super()._drain_and_barrier(tick_clock, wait_clock)
            if TAIL_MODE == 1:
                drain_inst = self.nc.sync.drain()
                wait_clock.add_sem_waits(
                    drain_inst.ins, ScopedClock({None: tick_clock.global_clock})
                )
                self.nc.all_engine_barrier(sem_only=True)
                popped = self.nc._tile_sem_poison_stack.pop()
                assert popped is self._sem_poison
                self.nc.clear_and_free_semaphores(
                    list(self.sems.allocated().values())
                )
                return
            # TAIL_MODE == 2: no in-kernel tail at all. In-body semaphores
            # already order every data dependency (incl. the output DMA);
            # NEFF completion itself waits for engine streams + DMA queues,
            # and the runtime epilogue zeroes the whole semaphore space.
            popped = self.nc._tile_sem_poison_stack.pop()
            assert popped is self._sem_poison

    return _FastTailTileContext(nc)


def _make_bacc():
    from concourse import bacc, mybir

    class _Bacc(bacc.Bacc):
        def __init__(self, *a, **kw):
            # Skip the const-memset all-engine barrier Bass.__init__ emits
            # (~1us on the critical path). The only consumer of those const
            # tiles here is the activation bias, which we replace with a
            # tile-tracked zero tile inside the TileContext.
            self._skip_init_barrier = True
            super().__init__(*a, **kw)
            self._skip_init_barrier = False

        def all_engine_barrier(self, *, sem_only: bool = False):
            if getattr(self, "_skip_init_barrier", False):
                return
            super().all_engine_barrier(sem_only=sem_only)

        def insert_act_table_loads(self):
            super().insert_act_table_loads()
            if not DROP_TABLE0:
                return
            # The entry-state fixpoint conservatively loads table set 0, but
            # every activation here is from the sigmoid set, which gets its
            # own load. Drop the set-0 load (1.28us on the ACT engine). Also
            # drop the const-* memsets whose only consumer (activation bias)
            # was replaced by an in-context zero tile.
            for blk in self.main_func.blocks:
                keep = []
                for inst in blk.instructions:
                    if (
                        isinstance(inst, mybir.InstLoadActFuncSet)
                        and inst.act_func_set_id == 0
                        and not (
                            inst.sync_info
                            and (inst.sync_info.on_wait or inst.sync_info.on_update)
                        )
                    ):
                        continue
                    if (
                        isinstance(inst, mybir.InstMemset)
                        and inst.outs
                        and str(inst.outs[0].memref).startswith("const-")
                        and not (
                            inst.sync_info
                            and (inst.sync_info.on_wait or inst.sync_info.on_update)
                        )
                    ):
                        continue
                    keep.append(inst)
                blk.instructions[:] = keep

    return _Bacc("TRN2", target_bir_lowering=False, debug=False, num_devices=M)


def _build_program(ncells_pad, chunks):
    from concourse import mybir

    f32 = mybir.dt.float32
    bf16 = mybir.dt.bfloat16
    Act = mybir.ActivationFunctionType
    Alu = mybir.AluOpType

    nc = _make_bacc()
    nchunks = len(chunks)

    NC = ncells_pad
    conf_t = nc.dram_tensor("conf", [P, FREE], bf16, kind="ExternalInput")
    # columns [0:NC]=u, [NC:2NC]=q, [2NC:3NC]=T; rows 24-127 are padding so
    # the DMA uses the fast full-128-partition descriptor path
    tin_t = nc.dram_tensor("tin", [P, 3 * NC], f32, kind="ExternalInput")
    oall_t = nc.dram_tensor("oall", [P, nchunks + 2], f32, kind="ExternalOutput")

    assert sum(chunks) == FREE

    with _make_tile_context(nc) as tc:
        with (
            tc.tile_pool(name="x", bufs=4) as xp,
            tc.tile_pool(name="s", bufs=3) as sp,
            tc.tile_pool(name="scr", bufs=2) as scrp,
            tc.tile_pool(name="acc", bufs=1) as accp,
            tc.tile_pool(name="tgt", bufs=1) as tp,
        ):
            acc = accp.tile([P, nchunks + 2], f32)

            # zero bias tile for every activation (replaces the global const
            # tile whose init barrier we skip; Tile orders the memset first)
            zb = accp.tile([P, 1], f32)
            nc.gpsimd.memset(zb[:], 0.0)

            # ---- masked cells: one (128, 3*NC) block, columns [u | q | T] ----
            t24 = tp.tile([P, 3 * NC], f32)
            nc.scalar.dma_start(t24[:], tin_t.ap()[:])
            # dummy first activation: binds the auto-inserted ACT table load
            # to the (early) zb memset instead of the part2 DMA receipt
            dum = tp.tile([P, 1], f32)
            nc.scalar.activation(dum[:], zb[:], Act.Sigmoid, bias=zb[:])
            sg = tp.tile([P, 2 * NC], f32)
            nc.scalar.activation(sg[:], t24[:, 0:2 * NC], Act.Sigmoid, bias=zb[:])
            rc = tp.tile([24, NC], f32)
            nc.vector.reciprocal_approx_fast(rc[:], sg[0:24, NC:2 * NC])
            fm = tp.tile([24, NC], f32)
            nc.vector.scalar_tensor_tensor(
                out=fm[:], in0=rc[:], scalar=-1.0, in1=sg[0:24, 0:NC],
                op0=Alu.add, op1=Alu.add)
            dm = tp.tile([24, NC], f32)
            nc.vector.scalar_tensor_tensor(
                out=dm[:], in0=fm[:], scalar=0.0, in1=t24[0:24, 2 * NC:3 * NC],
                op0=Alu.add, op1=Alu.subtract)
            t1 = tp.tile([24, NC], f32)
            nc.vector.scalar_tensor_tensor(
                out=t1[:], in0=dm[:], scalar=0.0, in1=dm[:],
                op0=Alu.add, op1=Alu.mult,
                accum_out=acc[0:24, nchunks:nchunks + 1])
            t2 = tp.tile([24, NC], f32)
            nc.vector.scalar_tensor_tensor(
                out=t2[:], in0=fm[:], scalar=0.0, in1=fm[:],
                op0=Alu.add, op1=Alu.mult,
                accum_out=acc[0:24, nchunks + 1:nchunks + 2])

            # ---- bulk: sum of sigmoid(conf)^2, chunked over the free dim ----
            # chunk 0's square runs on ACT (Square shares the sigmoid table)
            # to balance ACT vs DVE engine time; the rest square on DVE
            col = 0
            for i, cw in enumerate(chunks):
                x = xp.tile([P, cw], bf16, tag="x")
                nc.sync.dma_start(x[:], conf_t.ap()[:, col:col + cw])
                s = sp.tile([P, cw], bf16, tag="s")
                nc.scalar.activation(s[:], x[:], Act.Sigmoid, bias=zb[:])
                sq = scrp.tile([P, cw], bf16, tag="sq")
                nc.vector.scalar_tensor_tensor(
                    out=sq[:], in0=s[:], scalar=0.0, in1=s[:],
                    op0=Alu.add, op1=Alu.mult, accum_out=acc[:, i:i + 1])
                col += cw

            nc.sync.dma_start(oall_t.ap()[:], acc[:])

    nc.compile()
    return nc


def _get_program(ncells_pad, chunks):
    key = (ncells_pad, chunks)
    if key not in _PROGRAM_CACHE:
        _PROGRAM_CACHE[key] = _build_program(ncells_pad, chunks)
    return _PROGRAM_CACHE[key]


def kernel(pred, targets):
    global LAST
    from concourse.bass_utils import run_bass_kernel_spmd

    pred = np.ascontiguousarray(np.asarray(pred, dtype=np.float32))
    targets = np.asarray(targets, dtype=np.float32)
    assert pred.shape == (B, C, H, W), pred.shape
    N = targets.shape[0]

    # ---- host: parse targets, dedupe cells (last writer wins) ----
    b = targets[:, 0].astype(np.int32)
    c = targets[:, 1].astype(np.int32)
    gix = (targets[:, 2] * W).astype(np.int32)
    giy = (targets[:, 3] * H).astype(np.int32)
    valid = (gix < W) & (giy < H) & (gix >= 0) & (giy >= 0) & (b >= 0) & (b < B)

    cell_map = {}
    for i in range(N):
        if valid[i]:
            cell_map[(int(b[i]), int(giy[i]), int(gix[i]))] = i
    n_cells = len(cell_map)
    n = 3.0 * n_cells

    per_core = [[] for _ in range(M)]
    for (bb, yy, xx), i in cell_map.items():
        per_core[bb // BPC].append((bb, yy, xx, i))

    max_cells = max((len(pc) for pc in per_core), default=0)
    ncells_pad = max(32, ((max_cells + 31) // 32) * 32)

    # ---- host: build per-core shards ----
    pr = pred.reshape(B, A, 8, H, W)
    conf_all = pr[:, :, 4, :, :]  # (B, A, H, W)

    SIG_COL = np.array([k in (0, 1, 4, 5, 6, 7) for k in range(8)] * A)  # (24,)

    import ml_dtypes
    NC = ncells_pad
    in_maps = []
    for m in range(M):
        shard = np.ascontiguousarray(
            conf_all[m * BPC:(m + 1) * BPC]).reshape(P, FREE).astype(
                ml_dtypes.bfloat16)

        cells = per_core[m]
        tin = np.empty((P, 3 * NC), np.float32)
        tin[:, 0:NC] = NEG      # u pad -> sig = 0
        tin[:, NC:2 * NC] = -NEG  # q pad -> sig = 1 -> 1/sig - 1 = 0
        tin[:, 2 * NC:] = 0.0   # T pad
        if cells:
            bbs = np.array([e[0] for e in cells])
            yys = np.array([e[1] for e in cells])
            xxs = np.array([e[2] for e in cells])
            idx = np.array([e[3] for e in cells])
            vals = pred[bbs, :, yys, xxs].T  # (24, ncells)
            ncol = len(cells)
            tin[0:24, 0:ncol] = np.where(SIG_COL[:, None], vals, NEG)
            tin[0:24, NC:NC + ncol] = np.where(SIG_COL[:, None], -NEG, -vals)
            boxes = targets[idx, 2:6].T  # (4, ncells): gx, gy, gw, gh
            onehot = np.zeros((NUM_CLS, ncol), np.float32)
            ci = c[idx]
            ok = (ci >= 0) & (ci < NUM_CLS)
            onehot[ci[ok], np.nonzero(ok)[0]] = 1.0
            t0 = 2 * NC
            for a in range(A):
                tin[a * 8 + 0:a * 8 + 4, t0:t0 + ncol] = boxes
                tin[a * 8 + 4, t0:t0 + ncol] = 1.0
                tin[a * 8 + 5:a * 8 + 8, t0:t0 + ncol] = onehot
        in_maps.append({"conf": shard, "tin": tin})

    # ---- device ----
    nchunks = len(CHUNKS)
    nc = _get_program(ncells_pad, CHUNKS)
    res = run_bass_kernel_spmd(nc, in_maps, list(range(M)), trace=TRACE)
    LAST = res

    # ---- host: combine ----
    S2 = 0.0
    r1_tot = np.zeros(24, np.float64)
    r2_tot = np.zeros(24, np.float64)
    for m in range(M):
        out = res.results[m]["oall"].astype(np.float64)
        S2 += out[:, :nchunks].sum()
        r1_tot += out[0:24, nchunks]
        r2_tot += out[0:24, nchunks + 1]

    box_rows = [a * 8 + k for a in range(A) for k in range(4)]
    conf_rows = [a * 8 + 4 for a in range(A)]
    cls_rows = [a * 8 + k for a in range(A) for k in range(5, 8)]

    box_sum = r1_tot[box_rows].sum()
    cls_sum = r1_tot[cls_rows].sum()
    conf_corr = (r1_tot[conf_rows] - r2_tot[conf_rows]).sum()

    with np.errstate(divide="ignore", invalid="ignore"):
        loss_box = box_sum / (n * 4.0)
        loss_conf = (S2 + conf_corr) / float(B * A * HW)
        loss_cls = cls_sum / (n * NUM_CLS)
        total = 5.0 * loss_box + loss_conf + loss_cls
    return np.asarray(total, dtype=np.float32)
